# revision 114
# baseline (speedup 1.0000x reference)
"""BiLSTM classifier on 8 trn2 cores.

Sharding: 2 direction-groups x 4-way batch split (B_local=16).
Cores 0-3: forward direction, batches [0:16),[16:32),[32:48),[48:64).
Cores 4-7: backward direction, same batch slices, with time-reversed
inputs (a backward scan over x == forward scan over reversed x; the
masked SUM pooling is order-invariant so no un-reversal is needed).

Per-core program (identical SPMD program, different inputs):
  - embedding gather (indirect DMA) + PE transpose -> xt k-tiles in
    SBUF (chunk 0 is gathered host-side so the scan starts ~5us in);
    xt carries a ones-row so the W_ih bias rides in the tail k-tile.
  - 256-step LSTM scan: per step both gate PSUM banks accumulate the
    input projection (W_ih k-tile matmuls, xt-dependent, run early in
    the step's PE-idle window) plus 16 recurrent fp16 matmuls; bank A
    holds (i,f), bank B (o,g) so sigmoid(i,f) starts after only 8
    recurrent matmuls. fp32 cell state.
  - masked SUM pooling (mask broadcast to 128 partitions via
    ones-matmul, multiply+tree-reduce on the otherwise-idle GPSIMD
    engine every 16 steps) + half classifier -> partial logits [3,16].
Host sums fwd/bwd partials and applies mean-division + b_c.

The scan is latency-bound: the 1915ns/step critical chain is
h -> recurrent matmuls (+173ns PSUM drain) -> sigmoid(i,f) ->
tanh(g) (Act engine serializes) -> u -> c -> tanh(c) -> h, and every
hop sits at the cost model's fixed access/semaphore latencies. All
prep work is kept off that chain: each prep op carries a no-sync
scheduler edge anchored to the current step's chain (so the greedy
per-engine scheduler cannot slot a PSUM->SBUF copy ahead of a chain
op), chain ops get priority 0, and copies are split so no single op
exceeds the post-h DVE idle window.

Gate order within a step tile: X in (i, f, o, g).
"""

import os
from contextlib import ExitStack

import numpy as np

import concourse.bass as bass
import concourse.tile as tile
from concourse.tile_rust import add_dep_helper
from concourse import bacc, mybir
from concourse import masks as cmasks
from concourse.bass_utils import run_bass_kernel_spmd

F32 = mybir.dt.float32
F16 = mybir.dt.float16
I32 = mybir.dt.int32
AF = mybir.ActivationFunctionType
OP = mybir.AluOpType

V, E, H, C = 50000, 300, 256, 3
B = 64
NCORES = 8
BL = 16          # batch per core
HB = 2 * BL      # (hf, b) folded free width = 32
G4 = 4 * H       # 1024 gate rows
# permutation of pytorch gate-row order (i,f,g,o) -> kernel order (i,f,o,g)
GATE_PERM = np.r_[0:256, 256:512, 768:1024, 512:768]


# ---------------------------------------------------------------- host prep

def prep_in_maps(input_ids, attention_mask, emb, W_ih_f, W_hh_f, b_ih_f, b_hh_f,
                 W_ih_b, W_hh_b, b_ih_b, b_hh_b, W_c, b_c, T):
    emb_f16 = np.ascontiguousarray(np.asarray(emb, np.float16))
    in_maps = []
    for core in range(NCORES):
        d = core // 4          # 0 fwd, 1 bwd
        bs = slice((core % 4) * BL, (core % 4 + 1) * BL)
        ids = np.asarray(input_ids[bs], np.int32)[:, :T]
        msk = np.asarray(attention_mask[bs], np.float32)[:, :T]
        if d == 1:
            ids = ids[:, ::-1]
            msk = msk[:, ::-1]
        # t-major token order, [T*BL] -> lane-major [128, T*BL/128]
        ids_tb = np.ascontiguousarray(ids.T).reshape(-1)
        ids_in = np.ascontiguousarray(ids_tb.reshape(-1, 128).T)
        # pre-gathered+transposed x for the first two chunks (16 steps, 256
        # tokens): lets the scan start without waiting for the on-device
        # idx-DMA -> indirect-gather -> transpose chain.
        # row 300 = ones (pairs with the bias row of w_ihT's tail k-tile)
        x01 = np.concatenate(
            [emb_f16[ids_tb[:256]], np.ones((256, 1), np.float16)], 1)
        # maskrow[0, t*32 + hf*16 + b] = msk[b, t] (0/1, exact in f16; the
        # mean division and classifier bias are applied host-side)
        mT = np.ascontiguousarray(msk.T)                      # [T, BL]
        maskrow_n = np.ascontiguousarray(
            np.stack([mT, mT], axis=1).reshape(1, T * HB)).astype(np.float16)

        W_ih = (W_ih_f, W_ih_b)[d]
        W_hh = (W_hh_f, W_hh_b)[d]
        bias = (np.asarray(b_ih_f) + np.asarray(b_hh_f),
                np.asarray(b_ih_b) + np.asarray(b_hh_b))[d]
        W_ihp = np.asarray(W_ih, np.float32)[GATE_PERM].copy()  # [1024, 300]
        biasp = np.asarray(bias, np.float32)[GATE_PERM].copy()  # [1024]
        w_ihT = np.ascontiguousarray(
            np.concatenate([W_ihp.T, biasp[None, :]], 0).astype(np.float16))
        w_hhT = np.ascontiguousarray(
            np.asarray(W_hh, np.float32)[GATE_PERM].T.astype(np.float16))
        w_cT = np.ascontiguousarray(
            np.asarray(W_c, np.float32)[:, d * H:(d + 1) * H].T)  # [256, 3]
        in_maps.append({
            "ids": ids_in,
            "xt01": np.ascontiguousarray(x01.T),
            "maskrow": maskrow_n,
            "w_ihT": w_ihT,
            "w_hhT": w_hhT,
            "w_cT": w_cT,
            "emb": emb_f16,
        })
    return in_maps


def assemble(results, attention_mask, b_c):
    """Sum fwd/bwd partial (unnormalized) logits, apply the mean-pool
    division and classifier bias on host."""
    logits = np.zeros((B, C), np.float32)
    for core in range(NCORES):
        bs = slice((core % 4) * BL, (core % 4 + 1) * BL)
        logits[bs] += results[core]["out"].T
    cnt = np.clip(np.asarray(attention_mask, np.float32).sum(1), 1e-9, None)
    logits /= cnt[:, None]
    logits += np.asarray(b_c, np.float32)[None, :]
    return logits


# ---------------------------------------------------------------- kernel

def build_nc(T=256, debug=False):
    nc = bacc.Bacc("TRN2", target_bir_lowering=False, debug=debug,
                   num_devices=NCORES)
    ntok = T * BL
    NTT = ntok // 128             # 128-token tiles (32)

    ids_ap = nc.dram_tensor("ids", [128, NTT], I32, kind="ExternalInput").ap()
    xt01_ap = nc.dram_tensor("xt01", [E + 1, 256], F16, kind="ExternalInput").ap()
    maskrow_ap = nc.dram_tensor("maskrow", [1, T * HB], F16, kind="ExternalInput").ap()
    w_ihT_ap = nc.dram_tensor("w_ihT", [E + 1, G4], F16, kind="ExternalInput").ap()
    w_hhT_ap = nc.dram_tensor("w_hhT", [H, G4], F16, kind="ExternalInput").ap()
    w_cT_ap = nc.dram_tensor("w_cT", [H, C], F32, kind="ExternalInput").ap()
    emb_ap = nc.dram_tensor("emb", [V, E], F16, kind="ExternalInput").ap()
    out_ap = nc.dram_tensor("out", [C, BL], F32, kind="ExternalOutput").ap()

    EK = (128, 128, 44)           # E k-tile sizes
    EO = (0, 128, 256)
    BC = BL                       # batch cols per scan step tile
    SIG0_NS = 7200                # ~sim time of the first sigmoid (measured)

    with tile.TileContext(nc) as tc:
        with ExitStack() as octx:
            persist = octx.enter_context(tc.tile_pool(name="persist", bufs=1))
            hs = persist.tile([128, (T + 1) * HB], F16, tag="hs")
            idx_all = persist.tile([128, NTT], I32, tag="idx_all")
            wihA = persist.tile([128, 2 * G4], F16, tag="wihA")
            wih2b = persist.tile([EK[2] + 1, G4], F16, tag="wih2b")
            actprime = persist.tile([1, 2], F32, tag="actprime")
            whhA = persist.tile([128, 2 * G4], F16, tag="whhA")
            ident16 = persist.tile([128, 128], F16, tag="ident16")
            wcA = persist.tile([128, 2 * C], F32, tag="wcA")
            c0 = persist.tile([128, HB], F32, tag="c0")
            mb = persist.tile([128, T * HB], F16, tag="mb")
            mrow = persist.tile([1, T * HB], F16, tag="mrow")
            ones = persist.tile([1, 128], F16, tag="ones")

            # Prologue DMAs spread across SP(HWDGE), Act(HWDGE) and
            # gpsimd(SWDGE) queues — each issue costs >1us of queue time, and
            # the first scan step waits on wih/whh/x(chunk0).
            nc.sync.dma_start(
                wihA[:].rearrange("p (k c) -> p k c", k=2),
                w_ihT_ap[0:256, :].rearrange("(k p) c -> p k c", k=2))
            # prime the sigmoid/tanh activation table before the Act queue
            # fills with DMAs (the implicit table load costs 1283ns)
            nc.vector.memset(actprime[:], 0.0)
            nc.scalar.activation(actprime[:, 0:1], actprime[:, 0:1], AF.Sigmoid)
            nc.scalar.activation(actprime[:, 1:2], actprime[:, 1:2], AF.Tanh)
            # rows 256:301 of w_ihT: the 44-row tail k-tile plus the bias row
            nc.scalar.dma_start(wih2b[:], w_ihT_ap[EO[2]:E + 1, :])
            nc.gpsimd.dma_start(
                whhA[:].rearrange("p (k c) -> p k c", k=2),
                w_hhT_ap[:].rearrange("(k p) c -> p k c", k=2))
            cmasks.make_identity(nc, ident16[:])
            nc.vector.memset(c0[:], 0.0)
            nc.vector.memset(hs[:, 0:HB], 0.0)
            nc.vector.memset(ones[:], 1.0)

            def wih(k):
                if k < 2:
                    return wihA[:, k * G4:(k + 1) * G4]
                return wih2b[:]          # 44 E-rows + the bias row

            def whh(k):
                return whhA[:, k * G4:(k + 1) * G4]

            with ExitStack() as mp:
                xgp = mp.enter_context(tc.tile_pool(name="xg", bufs=8))
                xtp = mp.enter_context(tc.tile_pool(name="xt", bufs=3))
                tpp = mp.enter_context(
                    tc.tile_pool(name="tp", bufs=3, space="PSUM"))
                prp = mp.enter_context(
                    tc.tile_pool(name="prj", bufs=1, space="PSUM"))
                gp = mp.enter_context(
                    tc.tile_pool(name="gates", bufs=2, space="PSUM"))
                sp = mp.enter_context(tc.tile_pool(name="sig", bufs=3))
                cp = mp.enter_context(tc.tile_pool(name="cell", bufs=3))
                pp_pool = mp.enter_context(tc.tile_pool(name="pool", bufs=1))

                # chunk schedule: chunk 0 is host-pre-gathered (16 steps)
                sizes = [16, 16] + [32] * ((T - 32) // 32)
                assert sum(sizes) == T
                starts = [sum(sizes[:i]) for i in range(len(sizes))]
                chunks = list(zip(starts, sizes))
                pre_ch = {}

                gather_tp = {}
                anchors = {"mm": None, "h": None}

                def anch(inst, which):
                    """scheduler-only edge: this prep op may not be scheduled
                    before the current step's chain anchor (last recurrent
                    matmul for PE preps, the h-write for DVE preps)."""
                    a = anchors[which]
                    if a is not None and inst is not None:
                        add_dep_helper(
                            getattr(inst, "ins", inst), getattr(a, "ins", a),
                            sync=False, reason="prep-after-chain")

                def gather_fetch(t0, tt):
                    """gather 128 tokens (8 steps) + PE transpose into PSUM.
                    xg col 300 is a ones-column so the k=2 transpose emits the
                    ones-row that pairs with wih2b's bias row."""
                    g = (t0 * BL) // 128 + tt
                    xg = xgp.tile([128, E + 1], F16, tag="xg",
                                  name=f"xg{t0}_{tt}")
                    nc.gpsimd.indirect_dma_start(
                        out=xg[:, 0:E], out_offset=None, in_=emb_ap[:],
                        in_offset=bass.IndirectOffsetOnAxis(
                            ap=idx_all[:, g:g + 1], axis=0),
                    )
                    nc.gpsimd.memset(xg[:, E:E + 1], 1.0)
                    tp = tpp.tile([128, 3 * 128], F16, tag="tp",
                                  name=f"tp{t0}_{tt}")
                    for k in range(3):
                        ek = EK[k] + (1 if k == 2 else 0)
                        anch(nc.tensor.transpose(
                            tp[:ek, bass.ts(k, 128)],
                            xg[:, EO[k]:EO[k] + ek], ident16[:]), "mm")
                    gather_tp[(t0, tt)] = tp

                def gather_copy(t0, tt, k):
                    """PSUM->SBUF xt copy (deferred past the h write). One
                    k-tile per item so a pop stays well under the post-h
                    DVE-idle window."""
                    xt = pre_ch[t0]["xt"]
                    if k == 2:
                        tp = gather_tp.pop((t0, tt))
                    else:
                        tp = gather_tp[(t0, tt)]
                    ek = EK[k] + (1 if k == 2 else 0)
                    anch(nc.vector.tensor_copy(
                        xt[k][:ek, bass.ts(tt, 128)],
                        tp[:ek, bass.ts(k, 128)]), "h")

                def chunk_work(ci):
                    """typed work items ('pe'|'dve', closure) for chunk ci:
                    gather fetch+transpose (PE window) and PSUM->SBUF xt
                    copies (post-h DVE window). The input projection itself
                    happens inside front(t), accumulated straight into the
                    gate PSUM banks. Chunks 0/1 use host-pre-gathered x
                    (DMA'd directly, DVE queue) so the scan starts without
                    the on-device idx->gather->transpose chain."""
                    t0, ns = chunks[ci]
                    ntt = ns * BL // 128
                    xt = [xtp.tile([EK[k] + (1 if k == 2 else 0), ns * BL],
                                   F16, tag=f"xt{k}", name=f"xt{k}_{ci}")
                          for k in range(3)]
                    pre_ch[t0] = {"xt": xt}
                    if ci < 1 and os.environ.get("KHOSTX", "1") == "1":
                        # host-pre-gathered x, one DMA per k-tile, one per
                        # queue so no single DGE queue serializes the start
                        nc.sync.dma_start(xt[0][:], xt01_ap[0:128, :])
                        nc.scalar.dma_start(xt[1][:], xt01_ap[128:256, :])
                        nc.gpsimd.dma_start(xt[2][:], xt01_ap[256:E + 1, :])
                        return []
                    # all fetches ahead of all copies: a copy then pops >=2
                    # steps after its fetch, so the ~2.4us indirect-DMA
                    # latency is absorbed before the copy's anchored window
                    # (otherwise the copy slides into a later step's h slot)
                    items = []
                    for tt in range(ntt):
                        items.append(("pe", lambda tt=tt: gather_fetch(t0, tt)))
                    for tt in range(ntt):
                        for k in range(3):
                            items.append(("dve", lambda tt=tt, k=k:
                                          gather_copy(t0, tt, k)))
                    return items

                built_j = [0]
                mb_pb = {}

                def mb_mm(j):
                    pb = prp.tile([128, 512], F32, tag="prj", name=f"pb{j}")
                    mb_pb[j] = pb
                    for c0 in range(0, 512, 256):
                        anch(nc.tensor.matmul(
                            pb[:, c0:c0 + 256],
                            ones[:], mrow[:, j * 512 + c0:j * 512 + c0 + 256],
                            start=True, stop=True), "mm")

                def mb_copy(j, q):
                    pb = mb_pb[j] if q < 3 else mb_pb.pop(j)
                    c0 = q * 128
                    anch(nc.vector.tensor_copy(
                        mb[:, j * 512 + c0:j * 512 + c0 + 128],
                        pb[:, c0:c0 + 128]), "h")

                def mb_items(t1):
                    items = []
                    while built_j[0] * 512 < t1 * HB:
                        j = built_j[0]
                        items.append(("pe", lambda j=j: mb_mm(j)))
                        for q in range(4):
                            items.append(("dve", lambda j=j, q=q:
                                          mb_copy(j, q)))
                        built_j[0] += 1
                    return items

                st = {"c": c0[:, 0:2 * BC], "sig": None, "sigo": None, "cn": None}

                def front(t):
                    """gate matmuls in 2 psum banks + sig/tanh + c update.

                    bank A holds (i, f), bank B holds (o, g). The input
                    projection (W_ih k-tiles + bias) accumulates into the
                    same banks; those matmuls depend only on xt, so they
                    execute early in the step's PE-idle window. sigmoid(i,f)
                    issues after only 8 of the 16 recurrent matmuls."""
                    ck = max(i for i, (s, _) in enumerate(chunks) if s <= t)
                    t0 = chunks[ck][0]
                    xt = pre_ch[t0]["xt"]
                    tb = t - t0
                    pa = gp.tile([128, 4 * BC], F32, tag="ga", name="ga")
                    pb = gp.tile([128, 4 * BC], F32, tag="gb", name="gb")
                    for bank, x in [(pa, 0), (pa, 1), (pb, 2), (pb, 3)]:
                        for hf in range(2):
                            m = x * 2 + hf
                            col = bank[:, (x % 2) * 2 * BC + hf * BC:
                                       (x % 2) * 2 * BC + (hf + 1) * BC]
                            for k in range(3):
                                # exactly one start per bank: PSUM's zero
                                # region is the whole 2KB bank row. k==2
                                # includes the bias row times xt's ones-row.
                                nc.tensor.matmul(
                                    col, wih(k)[:, bass.ts(m, 128)],
                                    xt[k][:, tb * BL:(tb + 1) * BL],
                                    start=(x % 2 == 0 and hf == 0 and k == 0),
                                    stop=False)
                    for bank, x in [(pa, 0), (pa, 1), (pb, 2), (pb, 3)]:
                        for hf in range(2):
                            for k in range(2):
                                bank_last = (x % 2 == 1 and hf == 1 and k == 1)
                                mi = nc.tensor.matmul(
                                    bank[:, (x % 2) * 2 * BC + hf * BC:
                                         (x % 2) * 2 * BC + (hf + 1) * BC],
                                    whh(k)[:, x * 256 + hf * 128:x * 256 + (hf + 1) * 128],
                                    hs[:, t * HB + k * 16:t * HB + k * 16 + BC],
                                    start=False, stop=bank_last)
                    anchors["mm"] = mi
                    sig = sp.tile([128, 4 * BC], F16, tag="sig", name="sig")
                    nc.scalar.activation(sig[:], pa[:], AF.Sigmoid)
                    tg = sp.tile([128, 2 * BC], F16, tag="tg", name="tg")
                    nc.scalar.activation(tg[:], pb[:, 2 * BC:4 * BC], AF.Tanh)
                    sigo = sp.tile([128, 2 * BC], F16, tag="sigo", name="sigo")
                    nc.scalar.activation(sigo[:], pb[:, 0:2 * BC], AF.Sigmoid)
                    v = cp.tile([128, 2 * BC], F32, tag="v", name="v")
                    nc.vector.tensor_tensor(v[:], sig[:, 2 * BC:4 * BC],
                                            st["c"], OP.mult)
                    u = cp.tile([128, 2 * BC], F16, tag="u", name="u")
                    nc.vector.tensor_tensor(u[:], sig[:, 0:2 * BC], tg[:], OP.mult)
                    cn = cp.tile([128, 2 * BC], F32, tag="c", name="c")
                    nc.vector.tensor_tensor(cn[:], u[:], v[:], OP.add)
                    st["sig"], st["sigo"], st["cn"] = sig, sigo, cn

                def tail(t):
                    """h = sig_o * tanh(c)"""
                    sigo, cn = st["sigo"], st["cn"]
                    thc = sp.tile([128, 2 * BC], F16, tag="thc", name="thc")
                    nc.scalar.activation(thc[:], cn[:], AF.Tanh)
                    anchors["h"] = nc.vector.tensor_tensor(
                        hs[:, (t + 1) * HB:(t + 2) * HB],
                        sigo[:], thc[:], OP.mult)
                    st["c"] = cn

                PP = 16                     # steps per pooling piece
                parts = []

                def pool_piece(t0, n=PP, eng=None):
                    """masked partial sum of h over steps [t0, t0+n).

                    Runs on gpsimd (SBUF-only ops, idle engine) during the
                    scan; the final pieces run on DVE, which is idle once the
                    scan ends and has faster per-op latency."""
                    eng = eng or nc.gpsimd
                    mk = pp_pool.tile([128, PP * HB], F16, tag="mk",
                                      name=f"mk{t0}", bufs=2)
                    eng.tensor_tensor(
                        mk[:, :n * HB], hs[:, (t0 + 1) * HB:(t0 + n + 1) * HB],
                        mb[:, t0 * HB:(t0 + n) * HB], OP.mult)
                    # reduce over t as a binary tree of adds
                    # (gpsimd tensor_reduce can't reduce the free axis)
                    cur = mk
                    w = n * HB
                    while w > HB:
                        w //= 2
                        nxt = pp_pool.tile([128, w], F32, tag=f"mkr{w}",
                                           name=f"mkr{w}_{t0}", bufs=2)
                        eng.tensor_tensor(
                            nxt[:], cur[:, 0:w], cur[:, w:2 * w], OP.add)
                        cur = nxt
                    parts.append(cur)
                    if len(parts) >= 2:
                        a, b = parts.pop(), parts.pop()
                        s = pp_pool.tile([128, HB], F32, tag="psum",
                                         name=f"ps{t0}", bufs=2)
                        eng.tensor_tensor(s[:], a[:], b[:], OP.add)
                        parts.append(s)

                # ---------------- interleaved schedule
                #
                # Single strictly-ordered work queue, popped only from the
                # head. Each prep op carries a scheduler-only dependency edge
                # on the current step's chain anchor (last recurrent matmul
                # for PE work, the h write for DVE copies), so the greedy
                # per-engine scheduler can never run it ahead of the chain;
                # chain ops additionally get priority 0 so they win all ties.
                # Popping only from the head keeps issue order = queue order,
                # which the PSUM pool slab cycling (prj/tp tags) relies on.
                from collections import deque
                work = deque()
                for kind, fn in chunk_work(0):
                    fn()
                # idx/mask DMAs must issue before any reader issues (the
                # tile dep tracker orders by issue): mb_mm reads mrow, the
                # queued gather items read idx_all.
                nc.sync.dma_start(idx_all[:], ids_ap[:])
                nc.sync.dma_start(mrow[:], maskrow_ap[:])
                for kind, fn in mb_items(16):
                    fn()
                work.extend(chunk_work(1))
                work.extend(chunk_work(2))
                # cold DMAs, after the scan-critical prologue DMAs queued up
                nc.gpsimd.dma_start(
                    wcA[:].rearrange("p (k c) -> p k c", k=2),
                    w_cT_ap[:].rearrange("(k p) c -> p k c", k=2))
                next_chunk = 3
                for ci in range(len(chunks)):
                    t0, ns = chunks[ci]
                    if next_chunk < len(chunks):
                        work.extend(chunk_work(next_chunk))
                        next_chunk += 1
                    # build mask-broadcast blocks one chunk ahead so their
                    # copies never land on the busy chunk-boundary steps
                    work.extend(mb_items(min(T, t0 + ns + 32)))
                    for t in range(t0, t0 + ns):
                        with tc.high_priority():
                            front(t)
                            tail(t)
                        if t >= PP + PP // 2 and (t - PP // 2) % PP == 0 \
                                and t - PP - PP // 2 < T - PP:
                            pool_piece(t - PP - PP // 2)
                        if t == T - 5:
                            pool_piece(T - PP, PP // 2)
                        if t == T - 3:
                            pool_piece(T - PP // 2, PP // 4)
                        if t == T - 1:
                            pool_piece(T - 4, 2, eng=nc.vector)
                        npe = 2
                        ndve = 1
                        # pin prep work into this step's idle windows: 'pe'
                        # items into the post-recurrent-matmul stretch, 'dve'
                        # copies into the post-h window (the steady cadence
                        # is 1915ns/step, SIG0_NS ~= first sigmoid time)
                        ts0 = SIG0_NS + 1915 * t
                        while work and (npe or ndve):
                            kind = work[0][0]
                            if kind == "pe" and npe:
                                npe -= 1
                                off = -140
                            elif kind == "dve" and ndve:
                                ndve -= 1
                                off = 1460
                            else:
                                break
                            if os.environ.get("KWAIT", "0") == "1":
                                with tc.tile_wait_until(
                                        max(0, ts0 + off) / 1e6):
                                    work.popleft()[1]()
                            else:
                                work.popleft()[1]()
                assert not work, f"{len(work)} work items unissued"
                pool_piece(T - 2, 2, eng=nc.vector)

                # ---------------- tail: pooled -> logits
                # mask normalization is folded into mb (host-side), and the
                # classifier bias rides in as a K=1 matmul, so logits come
                # straight out of PSUM into the output DMA.
                while len(parts) > 1:
                    a, b = parts.pop(), parts.pop()
                    s = pp_pool.tile([128, HB], F32, tag="psum",
                                     name=f"fin{len(parts)}", bufs=2)
                    nc.vector.tensor_tensor(s[:], a[:], b[:], OP.add)
                    parts.append(s)
                pooled = parts[0]

                lg = prp.tile([C, BL], F32, tag="prj", name="lg")
                for k in range(2):
                    nc.tensor.matmul(lg[:], wcA[:, k * C:(k + 1) * C],
                                     pooled[:, k * BL:(k + 1) * BL],
                                     start=(k == 0), stop=(k == 1))
                ot = pp_pool.tile([C, BL], F32, tag="ot")
                nc.vector.tensor_copy(ot[:], lg[:])
                nc.sync.dma_start(out_ap[:], ot[:])

    nc.compile()
    return nc


# ---------------------------------------------------------------- entry

_NC_CACHE = {}


def kernel(**inputs) -> np.ndarray:
    """BiLSTM classifier forward on 8 trn2 NeuronCores.

    Takes the full unsharded inputs (as produced by setup_inputs()), runs
    the SPMD bass kernel on cores 0-7, returns full [64, 3] f32 logits.
    """
    T = 256
    if T not in _NC_CACHE:
        _NC_CACHE[T] = build_nc(T=T)
    nc = _NC_CACHE[T]
    np_inputs = {k: np.asarray(v) for k, v in inputs.items()}
    in_maps = prep_in_maps(T=T, **np_inputs)
    res = run_bass_kernel_spmd(nc, in_maps, list(range(NCORES)))
    return assemble(res.results, np_inputs["attention_mask"],
                    np_inputs["b_c"])


# revision 115
# speedup vs baseline: 1.0005x; 1.0005x over previous
"""BiLSTM classifier on 8 trn2 cores.

Sharding: 2 direction-groups x 4-way batch split (B_local=16).
Cores 0-3: forward direction, batches [0:16),[16:32),[32:48),[48:64).
Cores 4-7: backward direction, same batch slices, with time-reversed
inputs (a backward scan over x == forward scan over reversed x; the
masked SUM pooling is order-invariant so no un-reversal is needed).

Per-core program (identical SPMD program, different inputs):
  - embedding gather (indirect DMA) + PE transpose -> xt k-tiles in
    SBUF (chunk 0 is gathered host-side so the scan starts ~5us in);
    xt carries a ones-row so the W_ih bias rides in the tail k-tile.
  - 256-step LSTM scan: per step both gate PSUM banks accumulate the
    input projection (W_ih k-tile matmuls, xt-dependent, run early in
    the step's PE-idle window) plus 16 recurrent fp16 matmuls; bank A
    holds (i,f), bank B (o,g) so sigmoid(i,f) starts after only 8
    recurrent matmuls. fp32 cell state.
  - masked SUM pooling (mask broadcast to 128 partitions via
    ones-matmul, multiply+tree-reduce on the otherwise-idle GPSIMD
    engine every 16 steps) + half classifier -> partial logits [3,16].
Host sums fwd/bwd partials and applies mean-division + b_c.

The scan is latency-bound: the 1915ns/step critical chain is
h -> recurrent matmuls (+173ns PSUM drain) -> sigmoid(i,f) ->
tanh(g) (Act engine serializes) -> u -> c -> tanh(c) -> h, and every
hop sits at the cost model's fixed access/semaphore latencies. All
prep work is kept off that chain: each prep op carries a no-sync
scheduler edge anchored to the current step's chain (so the greedy
per-engine scheduler cannot slot a PSUM->SBUF copy ahead of a chain
op), chain ops get priority 0, and copies are split so no single op
exceeds the post-h DVE idle window.

Gate order within a step tile: X in (i, f, o, g).
"""

import os
from contextlib import ExitStack

import numpy as np

import concourse.bass as bass
import concourse.tile as tile
from concourse.tile_rust import add_dep_helper
from concourse import bacc, mybir
from concourse import masks as cmasks
from concourse.bass_utils import run_bass_kernel_spmd

F32 = mybir.dt.float32
F16 = mybir.dt.float16
I32 = mybir.dt.int32
AF = mybir.ActivationFunctionType
OP = mybir.AluOpType

V, E, H, C = 50000, 300, 256, 3
B = 64
NCORES = 8
BL = 16          # batch per core
HB = 2 * BL      # (hf, b) folded free width = 32
G4 = 4 * H       # 1024 gate rows
# permutation of pytorch gate-row order (i,f,g,o) -> kernel order (i,f,o,g)
GATE_PERM = np.r_[0:256, 256:512, 768:1024, 512:768]


# ---------------------------------------------------------------- host prep

def prep_in_maps(input_ids, attention_mask, emb, W_ih_f, W_hh_f, b_ih_f, b_hh_f,
                 W_ih_b, W_hh_b, b_ih_b, b_hh_b, W_c, b_c, T):
    emb_f16 = np.ascontiguousarray(np.asarray(emb, np.float16))
    in_maps = []
    for core in range(NCORES):
        d = core // 4          # 0 fwd, 1 bwd
        bs = slice((core % 4) * BL, (core % 4 + 1) * BL)
        ids = np.asarray(input_ids[bs], np.int32)[:, :T]
        msk = np.asarray(attention_mask[bs], np.float32)[:, :T]
        if d == 1:
            ids = ids[:, ::-1]
            msk = msk[:, ::-1]
        # t-major token order, [T*BL] -> lane-major [128, T*BL/128]
        ids_tb = np.ascontiguousarray(ids.T).reshape(-1)
        ids_in = np.ascontiguousarray(ids_tb.reshape(-1, 128).T)
        # pre-gathered+transposed x for the first two chunks (16 steps, 256
        # tokens): lets the scan start without waiting for the on-device
        # idx-DMA -> indirect-gather -> transpose chain.
        # row 300 = ones (pairs with the bias row of w_ihT's tail k-tile)
        x01 = np.concatenate(
            [emb_f16[ids_tb[:256]], np.ones((256, 1), np.float16)], 1)
        # maskrow[0, t*32 + hf*16 + b] = msk[b, t] (0/1, exact in f16; the
        # mean division and classifier bias are applied host-side)
        mT = np.ascontiguousarray(msk.T)                      # [T, BL]
        maskrow_n = np.ascontiguousarray(
            np.stack([mT, mT], axis=1).reshape(1, T * HB)).astype(np.float16)

        W_ih = (W_ih_f, W_ih_b)[d]
        W_hh = (W_hh_f, W_hh_b)[d]
        bias = (np.asarray(b_ih_f) + np.asarray(b_hh_f),
                np.asarray(b_ih_b) + np.asarray(b_hh_b))[d]
        W_ihp = np.asarray(W_ih, np.float32)[GATE_PERM].copy()  # [1024, 300]
        biasp = np.asarray(bias, np.float32)[GATE_PERM].copy()  # [1024]
        w_ihT = np.ascontiguousarray(
            np.concatenate([W_ihp.T, biasp[None, :]], 0).astype(np.float16))
        w_hhT = np.ascontiguousarray(
            np.asarray(W_hh, np.float32)[GATE_PERM].T.astype(np.float16))
        w_cT = np.ascontiguousarray(
            np.asarray(W_c, np.float32)[:, d * H:(d + 1) * H].T)  # [256, 3]
        in_maps.append({
            "ids": ids_in,
            "xt01": np.ascontiguousarray(x01.T),
            "maskrow": maskrow_n,
            "w_ihT": w_ihT,
            "w_hhT": w_hhT,
            "w_cT": w_cT,
            "emb": emb_f16,
        })
    return in_maps


def assemble(results, attention_mask, b_c):
    """Sum fwd/bwd partial (unnormalized) logits, apply the mean-pool
    division and classifier bias on host."""
    logits = np.zeros((B, C), np.float32)
    for core in range(NCORES):
        bs = slice((core % 4) * BL, (core % 4 + 1) * BL)
        logits[bs] += results[core]["out"].T
    cnt = np.clip(np.asarray(attention_mask, np.float32).sum(1), 1e-9, None)
    logits /= cnt[:, None]
    logits += np.asarray(b_c, np.float32)[None, :]
    return logits


# ---------------------------------------------------------------- kernel

def build_nc(T=256, debug=False):
    nc = bacc.Bacc("TRN2", target_bir_lowering=False, debug=debug,
                   num_devices=NCORES)
    ntok = T * BL
    NTT = ntok // 128             # 128-token tiles (32)

    ids_ap = nc.dram_tensor("ids", [128, NTT], I32, kind="ExternalInput").ap()
    xt01_ap = nc.dram_tensor("xt01", [E + 1, 256], F16, kind="ExternalInput").ap()
    maskrow_ap = nc.dram_tensor("maskrow", [1, T * HB], F16, kind="ExternalInput").ap()
    w_ihT_ap = nc.dram_tensor("w_ihT", [E + 1, G4], F16, kind="ExternalInput").ap()
    w_hhT_ap = nc.dram_tensor("w_hhT", [H, G4], F16, kind="ExternalInput").ap()
    w_cT_ap = nc.dram_tensor("w_cT", [H, C], F32, kind="ExternalInput").ap()
    emb_ap = nc.dram_tensor("emb", [V, E], F16, kind="ExternalInput").ap()
    out_ap = nc.dram_tensor("out", [C, BL], F32, kind="ExternalOutput").ap()

    EK = (128, 128, 44)           # E k-tile sizes
    EO = (0, 128, 256)
    BC = BL                       # batch cols per scan step tile
    SIG0_NS = 7200                # ~sim time of the first sigmoid (measured)

    with tile.TileContext(nc) as tc:
        with ExitStack() as octx:
            persist = octx.enter_context(tc.tile_pool(name="persist", bufs=1))
            hs = persist.tile([128, (T + 1) * HB], F16, tag="hs")
            idx_all = persist.tile([128, NTT], I32, tag="idx_all")
            wihA = persist.tile([128, 2 * G4], F16, tag="wihA")
            wih2b = persist.tile([EK[2] + 1, G4], F16, tag="wih2b")
            actprime = persist.tile([1, 2], F32, tag="actprime")
            whhA = persist.tile([128, 2 * G4], F16, tag="whhA")
            ident16 = persist.tile([128, 128], F16, tag="ident16")
            wcA = persist.tile([128, 2 * C], F32, tag="wcA")
            c0 = persist.tile([128, HB], F32, tag="c0")
            mb = persist.tile([128, T * HB], F16, tag="mb")
            mrow = persist.tile([1, T * HB], F16, tag="mrow")
            ones = persist.tile([1, 128], F16, tag="ones")

            # Prologue DMAs spread across SP(HWDGE), Act(HWDGE) and
            # gpsimd(SWDGE) queues — each issue costs >1us of queue time, and
            # the first scan step waits on wih/whh/x(chunk0).
            nc.sync.dma_start(
                wihA[:].rearrange("p (k c) -> p k c", k=2),
                w_ihT_ap[0:256, :].rearrange("(k p) c -> p k c", k=2))
            # prime the sigmoid/tanh activation table before the Act queue
            # fills with DMAs (the implicit table load costs 1283ns)
            nc.vector.memset(actprime[:], 0.0)
            nc.scalar.activation(actprime[:, 0:1], actprime[:, 0:1], AF.Sigmoid)
            nc.scalar.activation(actprime[:, 1:2], actprime[:, 1:2], AF.Tanh)
            # rows 256:301 of w_ihT: the 44-row tail k-tile plus the bias row
            nc.scalar.dma_start(wih2b[:], w_ihT_ap[EO[2]:E + 1, :])
            nc.gpsimd.dma_start(
                whhA[:].rearrange("p (k c) -> p k c", k=2),
                w_hhT_ap[:].rearrange("(k p) c -> p k c", k=2))
            cmasks.make_identity(nc, ident16[:])
            nc.vector.memset(c0[:], 0.0)
            nc.vector.memset(hs[:, 0:HB], 0.0)
            nc.vector.memset(ones[:], 1.0)

            def wih(k):
                if k < 2:
                    return wihA[:, k * G4:(k + 1) * G4]
                return wih2b[:]          # 44 E-rows + the bias row

            def whh(k):
                return whhA[:, k * G4:(k + 1) * G4]

            with ExitStack() as mp:
                xgp = mp.enter_context(tc.tile_pool(name="xg", bufs=8))
                xtp = mp.enter_context(tc.tile_pool(name="xt", bufs=3))
                tpp = mp.enter_context(
                    tc.tile_pool(name="tp", bufs=3, space="PSUM"))
                prp = mp.enter_context(
                    tc.tile_pool(name="prj", bufs=1, space="PSUM"))
                gp = mp.enter_context(
                    tc.tile_pool(name="gates", bufs=2, space="PSUM"))
                sp = mp.enter_context(tc.tile_pool(name="sig", bufs=3))
                cp = mp.enter_context(tc.tile_pool(name="cell", bufs=3))
                pp_pool = mp.enter_context(tc.tile_pool(name="pool", bufs=1))

                # chunk schedule: chunk 0 is host-pre-gathered (16 steps)
                sizes = [16, 16] + [32] * ((T - 32) // 32)
                assert sum(sizes) == T
                starts = [sum(sizes[:i]) for i in range(len(sizes))]
                chunks = list(zip(starts, sizes))
                pre_ch = {}

                gather_tp = {}
                anchors = {"mm": None, "h": None}

                def anch(inst, which):
                    """scheduler-only edge: this prep op may not be scheduled
                    before the current step's chain anchor (last recurrent
                    matmul for PE preps, the h-write for DVE preps)."""
                    a = anchors[which]
                    if a is not None and inst is not None:
                        add_dep_helper(
                            getattr(inst, "ins", inst), getattr(a, "ins", a),
                            sync=False, reason="prep-after-chain")

                def gather_fetch(t0, tt):
                    """gather 128 tokens (8 steps) + PE transpose into PSUM.
                    xg col 300 is a ones-column so the k=2 transpose emits the
                    ones-row that pairs with wih2b's bias row."""
                    g = (t0 * BL) // 128 + tt
                    xg = xgp.tile([128, E + 1], F16, tag="xg",
                                  name=f"xg{t0}_{tt}")
                    nc.gpsimd.indirect_dma_start(
                        out=xg[:, 0:E], out_offset=None, in_=emb_ap[:],
                        in_offset=bass.IndirectOffsetOnAxis(
                            ap=idx_all[:, g:g + 1], axis=0),
                    )
                    nc.gpsimd.memset(xg[:, E:E + 1], 1.0)
                    tp = tpp.tile([128, 3 * 128], F16, tag="tp",
                                  name=f"tp{t0}_{tt}")
                    for k in range(3):
                        ek = EK[k] + (1 if k == 2 else 0)
                        anch(nc.tensor.transpose(
                            tp[:ek, bass.ts(k, 128)],
                            xg[:, EO[k]:EO[k] + ek], ident16[:]), "mm")
                    gather_tp[(t0, tt)] = tp

                def gather_copy(t0, tt, k):
                    """PSUM->SBUF xt copy (deferred past the h write). One
                    k-tile per item so a pop stays well under the post-h
                    DVE-idle window."""
                    xt = pre_ch[t0]["xt"]
                    if k == 2:
                        tp = gather_tp.pop((t0, tt))
                    else:
                        tp = gather_tp[(t0, tt)]
                    ek = EK[k] + (1 if k == 2 else 0)
                    anch(nc.vector.tensor_copy(
                        xt[k][:ek, bass.ts(tt, 128)],
                        tp[:ek, bass.ts(k, 128)]), "h")

                def chunk_work(ci):
                    """typed work items ('pe'|'dve', closure) for chunk ci:
                    gather fetch+transpose (PE window) and PSUM->SBUF xt
                    copies (post-h DVE window). The input projection itself
                    happens inside front(t), accumulated straight into the
                    gate PSUM banks. Chunks 0/1 use host-pre-gathered x
                    (DMA'd directly, DVE queue) so the scan starts without
                    the on-device idx->gather->transpose chain."""
                    t0, ns = chunks[ci]
                    ntt = ns * BL // 128
                    xt = [xtp.tile([EK[k] + (1 if k == 2 else 0), ns * BL],
                                   F16, tag=f"xt{k}", name=f"xt{k}_{ci}")
                          for k in range(3)]
                    pre_ch[t0] = {"xt": xt}
                    if ci < 1 and os.environ.get("KHOSTX", "1") == "1":
                        # host-pre-gathered x, one DMA per k-tile, one per
                        # queue so no single DGE queue serializes the start
                        nc.sync.dma_start(xt[0][:], xt01_ap[0:128, :])
                        nc.scalar.dma_start(xt[1][:], xt01_ap[128:256, :])
                        nc.gpsimd.dma_start(xt[2][:], xt01_ap[256:E + 1, :])
                        return []
                    # all fetches ahead of all copies: a copy then pops >=2
                    # steps after its fetch, so the ~2.4us indirect-DMA
                    # latency is absorbed before the copy's anchored window
                    # (otherwise the copy slides into a later step's h slot)
                    items = []
                    for tt in range(ntt):
                        items.append(("pe", lambda tt=tt: gather_fetch(t0, tt)))
                    for tt in range(ntt):
                        for k in range(3):
                            items.append(("dve", lambda tt=tt, k=k:
                                          gather_copy(t0, tt, k)))
                    return items

                built_j = [0]
                mb_pb = {}

                def mb_mm(j):
                    pb = prp.tile([128, 512], F32, tag="prj", name=f"pb{j}")
                    mb_pb[j] = pb
                    for c0 in range(0, 512, 256):
                        anch(nc.tensor.matmul(
                            pb[:, c0:c0 + 256],
                            ones[:], mrow[:, j * 512 + c0:j * 512 + c0 + 256],
                            start=True, stop=True), "mm")

                def mb_copy(j, q):
                    pb = mb_pb[j] if q < 3 else mb_pb.pop(j)
                    c0 = q * 128
                    anch(nc.vector.tensor_copy(
                        mb[:, j * 512 + c0:j * 512 + c0 + 128],
                        pb[:, c0:c0 + 128]), "h")

                def mb_items(t1):
                    items = []
                    while built_j[0] * 512 < t1 * HB:
                        j = built_j[0]
                        items.append(("pe", lambda j=j: mb_mm(j)))
                        for q in range(4):
                            items.append(("dve", lambda j=j, q=q:
                                          mb_copy(j, q)))
                        built_j[0] += 1
                    return items

                st = {"c": c0[:, 0:2 * BC], "sig": None, "sigo": None, "cn": None}

                def front(t):
                    """gate matmuls in 2 psum banks + sig/tanh + c update.

                    bank A holds (i, f), bank B holds (o, g). The input
                    projection (W_ih k-tiles + bias) accumulates into the
                    same banks; those matmuls depend only on xt, so they
                    execute early in the step's PE-idle window. sigmoid(i,f)
                    issues after only 8 of the 16 recurrent matmuls."""
                    ck = max(i for i, (s, _) in enumerate(chunks) if s <= t)
                    t0 = chunks[ck][0]
                    xt = pre_ch[t0]["xt"]
                    tb = t - t0
                    pa = gp.tile([128, 4 * BC], F32, tag="ga", name="ga")
                    pb = gp.tile([128, 4 * BC], F32, tag="gb", name="gb")
                    for bank, x in [(pa, 0), (pa, 1), (pb, 2), (pb, 3)]:
                        for hf in range(2):
                            m = x * 2 + hf
                            col = bank[:, (x % 2) * 2 * BC + hf * BC:
                                       (x % 2) * 2 * BC + (hf + 1) * BC]
                            for k in range(3):
                                # exactly one start per bank: PSUM's zero
                                # region is the whole 2KB bank row. k==2
                                # includes the bias row times xt's ones-row.
                                nc.tensor.matmul(
                                    col, wih(k)[:, bass.ts(m, 128)],
                                    xt[k][:, tb * BL:(tb + 1) * BL],
                                    start=(x % 2 == 0 and hf == 0 and k == 0),
                                    stop=False)
                    for bank, x in [(pa, 0), (pa, 1), (pb, 2), (pb, 3)]:
                        for hf in range(2):
                            for k in range(2):
                                bank_last = (x % 2 == 1 and hf == 1 and k == 1)
                                mi = nc.tensor.matmul(
                                    bank[:, (x % 2) * 2 * BC + hf * BC:
                                         (x % 2) * 2 * BC + (hf + 1) * BC],
                                    whh(k)[:, x * 256 + hf * 128:x * 256 + (hf + 1) * 128],
                                    hs[:, t * HB + k * 16:t * HB + k * 16 + BC],
                                    start=False, stop=bank_last)
                    anchors["mm"] = mi
                    sig = sp.tile([128, 4 * BC], F16, tag="sig", name="sig")
                    nc.scalar.activation(sig[:], pa[:], AF.Sigmoid)
                    tg = sp.tile([128, 2 * BC], F16, tag="tg", name="tg")
                    nc.scalar.activation(tg[:], pb[:, 2 * BC:4 * BC], AF.Tanh)
                    sigo = sp.tile([128, 2 * BC], F16, tag="sigo", name="sigo")
                    nc.scalar.activation(sigo[:], pb[:, 0:2 * BC], AF.Sigmoid)
                    v = cp.tile([128, 2 * BC], F32, tag="v", name="v")
                    nc.vector.tensor_tensor(v[:], sig[:, 2 * BC:4 * BC],
                                            st["c"], OP.mult)
                    u = cp.tile([128, 2 * BC], F16, tag="u", name="u")
                    nc.vector.tensor_tensor(u[:], sig[:, 0:2 * BC], tg[:], OP.mult)
                    cn = cp.tile([128, 2 * BC], F32, tag="c", name="c")
                    nc.vector.tensor_tensor(cn[:], u[:], v[:], OP.add)
                    st["sig"], st["sigo"], st["cn"] = sig, sigo, cn

                def tail(t):
                    """h = sig_o * tanh(c)"""
                    sigo, cn = st["sigo"], st["cn"]
                    thc = sp.tile([128, 2 * BC], F16, tag="thc", name="thc")
                    nc.scalar.activation(thc[:], cn[:], AF.Tanh)
                    anchors["h"] = nc.vector.tensor_tensor(
                        hs[:, (t + 1) * HB:(t + 2) * HB],
                        sigo[:], thc[:], OP.mult)
                    st["c"] = cn

                PP = 16                     # steps per pooling piece
                parts = []

                def pool_piece(t0, n=PP, eng=None):
                    """masked partial sum of h over steps [t0, t0+n).

                    Runs on gpsimd (SBUF-only ops, idle engine) during the
                    scan; the final pieces run on DVE, which is idle once the
                    scan ends and has faster per-op latency."""
                    eng = eng or nc.gpsimd
                    mk = pp_pool.tile([128, PP * HB], F16, tag="mk",
                                      name=f"mk{t0}", bufs=2)
                    eng.tensor_tensor(
                        mk[:, :n * HB], hs[:, (t0 + 1) * HB:(t0 + n + 1) * HB],
                        mb[:, t0 * HB:(t0 + n) * HB], OP.mult)
                    # reduce over t as a binary tree of adds
                    # (gpsimd tensor_reduce can't reduce the free axis)
                    cur = mk
                    w = n * HB
                    while w > HB:
                        w //= 2
                        nxt = pp_pool.tile([128, w], F32, tag=f"mkr{w}",
                                           name=f"mkr{w}_{t0}", bufs=2)
                        eng.tensor_tensor(
                            nxt[:], cur[:, 0:w], cur[:, w:2 * w], OP.add)
                        cur = nxt
                    parts.append(cur)
                    if len(parts) >= 2:
                        a, b = parts.pop(), parts.pop()
                        s = pp_pool.tile([128, HB], F32, tag="psum",
                                         name=f"ps{t0}", bufs=2)
                        eng.tensor_tensor(s[:], a[:], b[:], OP.add)
                        parts.append(s)

                # ---------------- interleaved schedule
                #
                # Single strictly-ordered work queue, popped only from the
                # head. Each prep op carries a scheduler-only dependency edge
                # on the current step's chain anchor (last recurrent matmul
                # for PE work, the h write for DVE copies), so the greedy
                # per-engine scheduler can never run it ahead of the chain;
                # chain ops additionally get priority 0 so they win all ties.
                # Popping only from the head keeps issue order = queue order,
                # which the PSUM pool slab cycling (prj/tp tags) relies on.
                from collections import deque
                work = deque()
                for kind, fn in chunk_work(0):
                    fn()
                # idx/mask DMAs must issue before any reader issues (the
                # tile dep tracker orders by issue): mb_mm reads mrow, the
                # queued gather items read idx_all.
                nc.sync.dma_start(idx_all[:], ids_ap[:])
                nc.sync.dma_start(mrow[:], maskrow_ap[:])
                for kind, fn in mb_items(16):
                    fn()
                work.extend(chunk_work(1))
                work.extend(chunk_work(2))
                # cold DMAs, after the scan-critical prologue DMAs queued up
                nc.gpsimd.dma_start(
                    wcA[:].rearrange("p (k c) -> p k c", k=2),
                    w_cT_ap[:].rearrange("(k p) c -> p k c", k=2))
                next_chunk = 3
                for ci in range(len(chunks)):
                    t0, ns = chunks[ci]
                    if next_chunk < len(chunks):
                        work.extend(chunk_work(next_chunk))
                        next_chunk += 1
                    # build mask-broadcast blocks one chunk ahead so their
                    # copies never land on the busy chunk-boundary steps
                    work.extend(mb_items(min(T, t0 + ns + 32)))
                    for t in range(t0, t0 + ns):
                        with tc.high_priority():
                            front(t)
                            tail(t)
                        if t >= PP + PP // 2 and (t - PP // 2) % PP == 0 \
                                and t - PP - PP // 2 < T - PP:
                            pool_piece(t - PP - PP // 2)
                        if t == T - 5:
                            pool_piece(T - PP, PP // 2)
                        if t == T - 3:
                            pool_piece(T - PP // 2, PP // 4)
                        if t == T - 1:
                            pool_piece(T - 4, 2, eng=nc.vector)
                        npe = 1
                        ndve = 1
                        # pin prep work into this step's idle windows: 'pe'
                        # items into the post-recurrent-matmul stretch, 'dve'
                        # copies into the post-h window (the steady cadence
                        # is 1915ns/step, SIG0_NS ~= first sigmoid time)
                        ts0 = SIG0_NS + 1915 * t
                        while work and (npe or ndve):
                            kind = work[0][0]
                            if kind == "pe" and npe:
                                npe -= 1
                                off = -140
                            elif kind == "dve" and ndve:
                                ndve -= 1
                                off = 1460
                            else:
                                break
                            if os.environ.get("KWAIT", "0") == "1":
                                with tc.tile_wait_until(
                                        max(0, ts0 + off) / 1e6):
                                    work.popleft()[1]()
                            else:
                                work.popleft()[1]()
                assert not work, f"{len(work)} work items unissued"
                pool_piece(T - 2, 2, eng=nc.vector)

                # ---------------- tail: pooled -> logits
                # mask normalization is folded into mb (host-side), and the
                # classifier bias rides in as a K=1 matmul, so logits come
                # straight out of PSUM into the output DMA.
                while len(parts) > 1:
                    a, b = parts.pop(), parts.pop()
                    s = pp_pool.tile([128, HB], F32, tag="psum",
                                     name=f"fin{len(parts)}", bufs=2)
                    nc.vector.tensor_tensor(s[:], a[:], b[:], OP.add)
                    parts.append(s)
                pooled = parts[0]

                lg = prp.tile([C, BL], F32, tag="prj", name="lg")
                for k in range(2):
                    nc.tensor.matmul(lg[:], wcA[:, k * C:(k + 1) * C],
                                     pooled[:, k * BL:(k + 1) * BL],
                                     start=(k == 0), stop=(k == 1))
                ot = pp_pool.tile([C, BL], F32, tag="ot")
                nc.vector.tensor_copy(ot[:], lg[:])
                nc.sync.dma_start(out_ap[:], ot[:])

    nc.compile()
    return nc


# ---------------------------------------------------------------- entry

_NC_CACHE = {}


def kernel(**inputs) -> np.ndarray:
    """BiLSTM classifier forward on 8 trn2 NeuronCores.

    Takes the full unsharded inputs (as produced by setup_inputs()), runs
    the SPMD bass kernel on cores 0-7, returns full [64, 3] f32 logits.
    """
    T = 256
    if T not in _NC_CACHE:
        _NC_CACHE[T] = build_nc(T=T)
    nc = _NC_CACHE[T]
    np_inputs = {k: np.asarray(v) for k, v in inputs.items()}
    in_maps = prep_in_maps(T=T, **np_inputs)
    res = run_bass_kernel_spmd(nc, in_maps, list(range(NCORES)))
    return assemble(res.results, np_inputs["attention_mask"],
                    np_inputs["b_c"])
